# revision 5
# baseline (speedup 1.0000x reference)
"""TRN2 Bass kernel for nn_Brain: delayed-synapse recurrent network.

Strategy (dense delay-batched "futures", v3):
  total_input[t] = c0 + sum_{d=1}^{15} W_d @ acts_{t-d}   (acts_s, s>=1)
  acts_t = tanh(total_input[t])

- Edges with delay >= 16 never fire; delay-0 edges give a per-neuron
  constant c0 (computed on host, x64), deposited into PSUM by ONE matmul
  (lhsT = c0 packed [8, 128], rhs = one-hot [8, 128]) with start=True,
  which also zeroes the whole accumulator bank region.
- W_d dense [4096 src, 512 tgt] fp8e4m3 (x64) per core (8-way target
  shard); batch rows ride the weight stream as extra matmul columns.
- All apps accumulate straight into one PSUM tile holding all 16 step
  accumulators [128, (tc, t, r)] f32. Per step: tanh with fused 1/64
  scale off PSUM (ScalarE) -> DMA (Scalar HWDGE queue) -> AllGather ->
  landing DMA into the history tile.
- History landing avoids a 16B-descriptor scatter: the history keeps the
  AllGather-natural layout (128B contiguous per partition) and the source
  permutation is absorbed into the weight matrices on the host.
- Windows: bucket d applied in windows of nb <= d steps; windows with
  nb == d would sit on the next step's critical path (they need acts_t
  and feed step t+1), so they are split into (s0, nb-1) + (s0+nb-1, 1),
  leaving only the d=1 application on the per-step chain.
- d=1..7 SBUF-resident; d>=8 stream from HBM on the Sync HWDGE queue
  with tile prefetch several steps ahead. Per-step DMAs use the Scalar
  HWDGE queue so they never queue behind weight streams.
"""
import numpy as np

N_NEURONS = 4096
INPUT_SIZE = 1024
BATCH = 2
STEPS = 16
N_CORES = 8
TGT_PER_CORE = N_NEURONS // N_CORES        # 512
TCH = TGT_PER_CORE // 128                  # 4 target chunks per core
SCH = N_NEURONS // 128                     # 32 source chunks
MAXD = STEPS - 1                           # delays 1..15 useful
RESIDENT_D = (1, 2, 3, 4, 5, 6, 7)
FP8_SCALE = 64.0

_compiled = None


def _windows():
    """(d, s0, nb): contributes to steps [s0+d, s0+d+nb-1] from acts
    s0..s0+nb-1. Zero-slack windows (nb == d, d >= 2) split so only d=1
    rides the per-step critical chain."""
    apps = []
    for d in range(1, MAXD + 1):
        nsteps = STEPS - d
        nwin = -(-nsteps // d)
        base, extra = divmod(nsteps, nwin)
        s0 = 1
        for i in range(nwin):
            nb = base + (1 if i < extra else 0)
            if d >= 2 and nb == d:
                apps.append((d, s0, nb - 1))
                apps.append((d, s0 + nb - 1, 1))
            else:
                apps.append((d, s0, nb))
            s0 += nb
    return apps


def _build_program():
    from concourse import bacc, mybir, tile

    dt = mybir.dt
    nc = bacc.Bacc(None, target_bir_lowering=False, debug=False)

    wd_in = {}
    for d in range(1, MAXD + 1):
        wd_in[d] = nc.declare_dram_parameter(
            f"wd{d}", [128, SCH * TCH * 128], dt.float8e4, isOutput=False)
    c0k_in = nc.declare_dram_parameter("c0k", [8, 128], dt.bfloat16,
                                       isOutput=False)
    oh_in = nc.declare_dram_parameter("oh", [8, TCH * STEPS * BATCH],
                                      dt.bfloat16, isOutput=False)
    out_d = nc.declare_dram_parameter("out", [128, TCH * BATCH], dt.float32,
                                      isOutput=True)

    cc_in = nc.dram_tensor("cc_in", [128, TCH * BATCH], dt.bfloat16)
    cc_out = nc.dram_tensor("cc_out", [N_CORES * 128, TCH * BATCH],
                            dt.bfloat16, addr_space="Shared")

    apps = _windows()
    # Load-balanced apply times: app (d, s0, nb) may run at any step in
    # [s0+nb-1, s0+d-1]; assign greedily (earliest deadline first) to the
    # least-loaded feasible step. Streamed buckets (d>=8) wait 2 extra
    # steps at the front so their HBM prefetch leads the matmuls.
    load = {s: 0 for s in range(1, STEPS)}
    assign = []
    for (d, s0, nb) in sorted(apps, key=lambda a: (a[1] + a[0] - 1, a[0])):
        lo = s0 + nb - 1
        if d >= 9:
            lo = max(lo, 16 - d + 2)
        hi = min(s0 + d - 1, STEPS - 1)
        k_apply = min(range(lo, hi + 1), key=lambda s: (load[s], s))
        load[k_apply] += 1
        assign.append((k_apply, d, s0, nb))
    ready = {s: [] for s in range(0, STEPS + 1)}
    for (k_apply, d, s0, nb) in assign:
        ready[k_apply].append((d, s0, nb))
    for k in ready:
        ready[k].sort(key=lambda a: (a[0] != 1, a[0]))  # d=1 first

    # stream-tile prefetch: d=15..11 at program start (5 bufs), the rest
    # as tiles free up (d freed right after its last app)
    stream_d = [d for d in range(8, MAXD + 1)]
    last_app_step = {d: max(k for k, lst in ready.items()
                            for (dd, _, _) in lst if dd == d)
                     for d in stream_d}
    prefetch0 = [15, 14, 13, 12, 11]
    prefetch_at = {}                 # step -> [d]
    free_order = sorted(prefetch0, key=lambda d: last_app_step[d])
    for i, d in enumerate([10, 9, 8]):
        step = last_app_step[free_order[i]]
        prefetch_at.setdefault(step, []).append(d)

    HCOLS = MAXD * SCH * BATCH  # history cols: (s, ch, r), ch permuted

    with tile.TileContext(nc) as tc:
        with (
            tc.tile_pool(name="wres", bufs=1) as wres_pool,
            tc.tile_pool(name="wstream", bufs=5) as wstream_pool,
            tc.tile_pool(name="aux", bufs=1) as aux_pool,
            tc.tile_pool(name="psum", bufs=1, space="PSUM") as psum_pool,
        ):
            t_wres = {}
            for d in RESIDENT_D:
                t_wres[d] = wres_pool.tile([128, SCH * TCH * 128], dt.float8e4,
                                           name=f"wres{d}", tag=f"wres{d}")
            t_hist = aux_pool.tile([128, HCOLS], dt.bfloat16)
            t_c0k = aux_pool.tile([8, 128], dt.bfloat16)
            t_oh = aux_pool.tile([8, TCH * STEPS * BATCH], dt.bfloat16)
            t_act = aux_pool.tile([128, TCH * BATCH], dt.float32)
            t_actb = aux_pool.tile([128, TCH * BATCH], dt.bfloat16)
            t_acc = psum_pool.tile([128, TCH * STEPS * BATCH], dt.float32,
                                   name="acc", tag="acc")
            acc4 = t_acc[:].rearrange("p (tcch t r) -> p tcch t r",
                                      tcch=TCH, t=STEPS)

            nc.sync.dma_start(t_c0k[:], c0k_in[:])
            nc.sync.dma_start(t_oh[:], oh_in[:])
            nc.sync.dma_start(t_wres[1][:], wd_in[1][:])
            nc.sync.dma_start(t_wres[2][:], wd_in[2][:])
            t_wstream = {}
            for d in prefetch0:
                t_w = wstream_pool.tile([128, SCH * TCH * 128], dt.float8e4,
                                        name=f"wstr{d}", tag="wstream")
                nc.sync.dma_start(t_w[:], wd_in[d][:])
                t_wstream[d] = t_w
            for d in RESIDENT_D[2:]:
                nc.sync.dma_start(t_wres[d][:], wd_in[d][:])

            # c0 into all step columns + zero the accumulator (one matmul)
            nc.tensor.matmul(t_acc[:], t_c0k[:], t_oh[:],
                             start=True, stop=False, skip_group_check=True)

            def run_app(d, s0, nb, last):
                t_w = t_wres[d] if d in RESIDENT_D else (
                    t_wstream.pop(d) if last else t_wstream[d])
                w3 = t_w[:].rearrange("p (sc tcch m) -> p sc tcch m",
                                      sc=SCH, tcch=TCH)
                t0 = s0 + d
                for tc_i in range(TCH):
                    acc_win = acc4[:, tc_i, t0 - 1:t0 - 1 + nb, :]
                    for sc in range(SCH):
                        rhs = t_hist[:].rearrange(
                            "p (s c r) -> p s c r", s=MAXD, c=SCH
                        )[:, s0 - 1:s0 - 1 + nb, sc, :]
                        nc.tensor.matmul(
                            acc_win, w3[:, sc, tc_i, :], rhs,
                            start=False, stop=False, skip_group_check=True)

            napps_left = {d: sum(1 for (dd, _, _) in apps if dd == d)
                          for d in range(1, MAXD + 1)}

            for t in range(1, STEPS + 1):
                sc_ctx = nc.named_scope(f"step{t:02d}")
                sc_ctx.__enter__()
                acc_t = acc4[:, :, t - 1, :]
                if t == STEPS:
                    nc.scalar.activation(
                        t_act[:].rearrange("p (tcch r) -> p tcch r", tcch=TCH),
                        acc_t, mybir.ActivationFunctionType.Tanh,
                        scale=1.0 / FP8_SCALE)
                    nc.scalar.dma_start(out_d[:], t_act[:])
                    sc_ctx.__exit__(None, None, None)
                    break
                nc.scalar.activation(
                    t_actb[:].rearrange("p (tcch r) -> p tcch r", tcch=TCH),
                    acc_t, mybir.ActivationFunctionType.Tanh,
                    scale=1.0 / FP8_SCALE)
                nc.scalar.dma_start(cc_in[:], t_actb[:])
                nc.gpsimd.collective_compute(
                    "AllGather", mybir.AluOpType.bypass,
                    replica_groups=[list(range(N_CORES))],
                    ins=[cc_in[:]], outs=[cc_out[:]])
                # landing: contiguous 64 cols per partition (source
                # permutation absorbed into the weights host-side)
                src_ap = cc_out[:].rearrange("(pp e) c -> pp (e c)", e=8)
                dst_ap = t_hist[:].rearrange(
                    "p (s cr) -> p s cr", s=MAXD)[:, t - 1, :]
                nc.scalar.dma_start(dst_ap, src_ap)
                sc_ctx.__exit__(None, None, None)
                for d in prefetch_at.get(t, []):
                    t_w = wstream_pool.tile([128, SCH * TCH * 128],
                                            dt.float8e4, name=f"wstr{d}",
                                            tag="wstream")
                    nc.sync.dma_start(t_w[:], wd_in[d][:])
                    t_wstream[d] = t_w
                for (d, s0, nb) in ready.get(t, []):
                    napps_left[d] -= 1
                    with nc.named_scope(f"app_d{d}_s{s0}"):
                        run_app(d, s0, nb, napps_left[d] == 0)

    nc.compile()
    return nc


def _preprocess(input_data, connection_weights, connection_indices,
                delay_values, steps):
    """Host: per-core dense bucketed weights (source-permuted), c0."""
    assert steps == STEPS
    w = np.asarray(connection_weights, np.float32)
    ci = np.asarray(connection_indices)
    dl = np.asarray(delay_values)
    src, tgt = ci[0].astype(np.int64), ci[1].astype(np.int64)
    x = np.asarray(input_data, np.float32)

    acts0 = np.zeros((BATCH, N_NEURONS), np.float32)
    acts0[:, :INPUT_SIZE] = x

    m0 = dl == 0
    c0 = np.zeros((BATCH, N_NEURONS), np.float32)
    for r in range(BATCH):
        np.add.at(c0[r], tgt[m0], w[m0] * acts0[r, src[m0]])

    import ml_dtypes
    wds = {}
    for d in range(1, MAXD + 1):
        md = dl == d
        Wd = np.zeros((N_NEURONS, N_NEURONS), np.float32)
        np.add.at(Wd, (src[md], tgt[md]), w[md])
        wds[d] = Wd

    # history-landing permutation: hist[p', (sub, tc, r)] holds neuron
    # n = 512*(row//128) + 128*tc + row%128, row = 8*p' + sub
    pp = np.arange(128)[:, None, None]
    sub = np.arange(8)[None, :, None]
    tcc = np.arange(4)[None, None, :]
    row = 8 * pp + sub
    nperm = (512 * (row // 128) + 128 * tcc + row % 128)   # [128, 8, 4]
    nperm = nperm.reshape(128, SCH)                        # ch = sub*4+tc

    # one-hot rhs for the c0-deposit matmul
    oh = np.zeros((8, TCH, STEPS, BATCH), np.float32)
    for tci in range(TCH):
        for r in range(BATCH):
            oh[tci * BATCH + r, tci, :, r] = 1.0
    oh = oh.reshape(8, TCH * STEPS * BATCH).astype(ml_dtypes.bfloat16)

    in_maps = []
    for k in range(N_CORES):
        t0, t1 = k * TGT_PER_CORE, (k + 1) * TGT_PER_CORE
        im = {"oh": oh}
        for d in range(1, MAXD + 1):
            Ws = wds[d][:, t0:t1]                          # [4096, 512]
            Wp = Ws[nperm, :]                              # [128, 32, 512]
            Wp = Wp.reshape(128, SCH * TCH * 128)
            im[f"wd{d}"] = (Wp * FP8_SCALE).astype(ml_dtypes.float8_e4m3fn)
        c0k = np.zeros((8, 128), np.float32)
        for tci in range(TCH):
            for r in range(BATCH):
                c0k[tci * BATCH + r] = c0[r, t0 + tci * 128:
                                          t0 + (tci + 1) * 128]
        im["c0k"] = (c0k * FP8_SCALE).astype(ml_dtypes.bfloat16)
        in_maps.append(im)
    return in_maps


def kernel(input_data, connection_weights, connection_indices,
           delay_values, steps):
    global _compiled
    from concourse.bass_utils import run_bass_kernel_spmd

    in_maps = _preprocess(input_data, connection_weights,
                          connection_indices, delay_values, int(steps))
    if _compiled is None:
        _compiled = _build_program()
    res = run_bass_kernel_spmd(_compiled, in_maps, list(range(N_CORES)))

    out = np.zeros((BATCH, N_NEURONS), np.float32)
    for k in range(N_CORES):
        o = res.results[k]["out"]
        t0 = k * TGT_PER_CORE
        for tci in range(TCH):
            for r in range(BATCH):
                out[r, t0 + tci * 128: t0 + (tci + 1) * 128] = \
                    o[:, tci * BATCH + r]
    return out[:, -INPUT_SIZE:].astype(np.float32)
